# revision 20
# baseline (speedup 1.0000x reference)
"""KAN layer on 8 Trainium2 NeuronCores.

Reference computation (fp32):
    basis[t, i, n, o] = tanh(h[i, n, o] * x[t, i] + b[i, n, o])
    out[t, o]         = sum_{i,n} basis[t, i, n, o] * w[i, n, o]
with B,S,I,N,O = 2,1024,64,16,64 and t = (batch, seq) flattened to 2048 tokens.

Strategy (o-shard, SPMD on 8 cores):
 - Each core owns 8 of the 64 output channels and the full 2048-token stream.
 - SBUF layout puts 128 (n_sub, i) pairs on partitions (n = 2*c + n_sub, c in
   0..7 chunks), tokens on the free dim. x^T is replicated onto both partition
   halves once, so ONE activation instruction per (chunk, o) computes
   tanh(h_col * x + b_col) for 128 (i,n) pairs x 2048 tokens, with h/b as
   per-partition scale/bias operands (the ACT affine stage is free).
 - The (i,n) contraction with w is 256 tiny PE matmuls (stationary w column
   [128,1], moving basis [128,512] bf16) accumulating fp32 in PSUM across the
   8 chunks; results DMA straight from PSUM to DRAM as an [8, 2048] o-major
   slab per core. Host concatenates, transposes, reshapes.

ACT is the bound: 64 instrs x (2048+352)/1.2GHz ~ 128us/core.
"""

import numpy as np

import concourse.bass as bass
import concourse.bacc as bacc
import concourse.tile as tile
from concourse import mybir
from concourse.bass_utils import run_bass_kernel_spmd

B, S, I, N, O = 2, 1024, 64, 16, 64
T = B * S              # 2048 tokens
NCORES = 8
OL = O // NCORES       # 8 output channels per core
CH = N // 2            # 8 chunks of n-pairs; partitions = (n_sub:2, i:64) = 128
TQ = 4                 # token quarters -> 512-wide matmuls (one PSUM bank)
TQW = T // TQ

_cache = {}


def _build():
    # Bacc (not raw Bass): its compile() runs generate_event_semaphores,
    # which splits multi-wait sync onto EventSemaphore instructions to
    # satisfy TRN2's one-wait-per-instruction limit (the final Tile drain
    # carries a wait per semaphore and needs this).
    nc = bacc.Bacc()
    f32 = mybir.dt.float32
    bf16 = mybir.dt.bfloat16

    PW = CH * OL  # 64 param columns per tensor
    # Single packed input [x^T(dup) | h | b | w]: ONE DMA, so every consumer
    # waits on a single DMA-queue semaphore (TRN2 ACT queue holds 1 wait).
    XW = T + 3 * PW
    xprm = nc.declare_dram_parameter("xprm", [128, XW], f32, isOutput=False)
    out = nc.declare_dram_parameter("o", [OL, T], f32, isOutput=True)

    with tile.TileContext(nc) as tc:
        with (
            tc.tile_pool(name="const", bufs=1) as cpool,
            tc.tile_pool(name="basis", bufs=3) as bpool,
            tc.tile_pool(name="ps", bufs=8, space="PSUM") as ppool,
            tc.tile_pool(name="stage", bufs=8) as spool,
        ):
            xp_sb = cpool.tile([128, XW], f32, tag="xprm")
            w_bf = cpool.tile([128, PW], bf16, tag="wbf")
            scratch = cpool.tile([1, 1], f32, tag="scr")
            xrep = xp_sb[:, 0:T]
            h_sb = xp_sb[:, T:T + PW]
            b_sb = xp_sb[:, T + PW:T + 2 * PW]

            # SWDGE for the input so the 8 HWDGE queues are left exclusively
            # to the 8 output DMAs (a 9th HWDGE descriptor would wrap onto
            # queue 0 and need a second, unsupported queue-order wait).
            nc.gpsimd.dma_start(xp_sb[:], xprm[:])
            nc.vector.tensor_copy(w_bf[:], xp_sb[:, T + 2 * PW:T + 3 * PW])
            # Touch tanh immediately so the ~2.7us ACT table load starts as
            # soon as the input DMA lands.
            nc.scalar.activation(
                scratch[:], xp_sb[0:1, 0:1], mybir.ActivationFunctionType.Tanh
            )

            for ol in range(OL):
                psums = [
                    ppool.tile([1, TQW], f32, tag="ps", name=f"ps_{ol}_{tq}")
                    for tq in range(TQ)
                ]
                for c in range(CH):
                    col = c * OL + ol
                    basis = bpool.tile([128, T], bf16, tag="basis")
                    nc.scalar.activation(
                        basis[:],
                        xrep[:],
                        mybir.ActivationFunctionType.Tanh,
                        bias=b_sb[:, col:col + 1],
                        scale=h_sb[:, col:col + 1],
                    )
                    for tq in range(TQ):
                        nc.tensor.matmul(
                            psums[tq][:],
                            lhsT=w_bf[:, col:col + 1],
                            rhs=basis[:, bass.ts(tq, TQW)],
                            start=(c == 0),
                            stop=(c == CH - 1),
                        )
                # PE wrote each [1, 512] result on partition 0 of its PSUM
                # bank; DVE evicts in-partition to an SBUF staging row and
                # the DMA does the cross-partition placement into row ol.
                stage = spool.tile([1, T], f32, tag="stage", name=f"stage_{ol}")
                for tq in range(TQ):
                    nc.vector.tensor_copy(
                        stage[:, bass.ts(tq, TQW)], psums[tq][:]
                    )
                nc.sync.dma_start(out[ol:ol + 1, :], stage[:])
                # Sacrificial [1,4] weight load that alone carries the
                # PE-waits-on-DVE edge for PSUM bank reuse, so the next
                # accumulation group's matmul keeps a single (ACT) wait —
                # the TRN2 MM queue descriptor holds one wait command.
                # ldweights can't take fp32, so bounce one element per
                # evicted slice through a bf16 signal tile (the DVE copy
                # needs no wait of its own: same-engine FIFO after the
                # evictions). Clobbered stationary state is fine: every
                # matmul reloads its own lhsT.
                sig = spool.tile([1, TQ], bf16, tag="sig", name=f"sig_{ol}")
                nc.vector.tensor_copy(sig[:], stage[0:1, 0:T:TQW])
                nc.tensor.ldweights(sig[:])

    _strip_self_waits(nc)
    # Run Bacc's compile pipeline (register allocation, nop fusion, and
    # generate_event_semaphores wait legalization) before serialization.
    nc.finalize()
    return nc


# Compute instructions on in-order engines never need to wait on their own
# engine's completion semaphore: ACT/DVE execute strictly in order, and PE
# MATMULs are pc-monotone in start and end (the 64-deep window only pulls
# LDWEIGHTS ahead, which here only ever reads the write-once w_bf tile).
# Tile emits these self-waits conservatively, but TRN2 queue descriptors
# hold a single wait command, so dropping the provably-satisfied self-wait
# keeps each instruction within hardware limits.
_STRIPPABLE = {"InstActivation", "InstTensorCopy", "InstTensorTensor",
               "InstTensorScalarPtr", "InstTensorReduce", "InstMemSet",
               "InstMatmult", "InstLdWeights"}
_ENG_PREFIX = {"Activation": "Activation_", "DVE": "DVE_", "PE": "PE_"}


def _strip_self_waits(nc):
    for bb in nc.main_func.blocks:
        for ins in bb.instructions:
            if type(ins).__name__ not in _STRIPPABLE:
                continue
            eng = str(ins.engine).split(".")[-1]
            pfx = _ENG_PREFIX.get(eng)
            si = ins.sync_info
            if pfx is None or si is None or len(si.on_wait) < 2:
                continue
            kept = [w for w in si.on_wait if not w.ant_name.startswith(pfx)]
            if len(kept) != len(si.on_wait):
                si.on_wait = kept
                ins.sync_info = si


def _shuffle(p, k):
    """[I, N, O] param -> core k's [128, CH*OL] SBUF layout.

    row = n_sub*64 + i  (n = 2*c + n_sub), col = c*OL + ol (o = k*OL + ol).
    """
    sl = p[:, :, k * OL:(k + 1) * OL]                     # [I, N, OL]
    return np.ascontiguousarray(
        sl.reshape(I, CH, 2, OL).transpose(2, 0, 1, 3).reshape(128, CH * OL)
    )


def _prep(x, w, h, b):
    xt = x.reshape(T, I).T                                # [I, T]
    xt2 = np.concatenate([xt, xt], axis=0)                # [128, T]
    return [
        {
            "xprm": np.ascontiguousarray(
                np.concatenate(
                    [xt2, _shuffle(h, k), _shuffle(b, k), _shuffle(w, k)],
                    axis=1,
                )
            )
        }
        for k in range(NCORES)
    ]


def _gather(results):
    outT = np.concatenate([results[k]["o"] for k in range(NCORES)], axis=0)  # [O, T]
    return np.ascontiguousarray(outT.T).reshape(B, S, O).astype(np.float32)


def _run(x, w, h, b, **kwargs):
    if "nc" not in _cache:
        _cache["nc"] = _build()
    in_maps = _prep(
        np.asarray(x, np.float32),
        np.asarray(w, np.float32),
        np.asarray(h, np.float32),
        np.asarray(b, np.float32),
    )
    return run_bass_kernel_spmd(_cache["nc"], in_maps, list(range(NCORES)), **kwargs)


def kernel(x, w, h, b):
    return _gather(_run(x, w, h, b).results)


def bench(x, w, h, b, **trace_kwargs):
    """Run with NTFF profiling; returns (output, BassKernelResults)."""
    br = _run(x, w, h, b, trace=True, **trace_kwargs)
    return _gather(br.results), br
